# revision 1
# baseline (speedup 1.0000x reference)
"""GPT-2 small forward pass on 8 Trainium2 NeuronCores (Bass/Tile).

Sharding: sequence-parallel trunk (cores 0-3 batch 0, cores 4-7 batch 1;
core owns 2 contiguous 128-token blocks), per-layer AllGather of K/V (bf16)
within each 4-core group, then vocab-sharded lm_head (8 ways) after an
8-core AllGather of the final LN output.

Layout: activations feature-major xT [768, tok] on-chip; scores computed
transposed (K stationary), PV uses token-major V with an appended ones
column so softmax denominators fall out of the same matmul; Z division is
folded into the PV epilogue. No PE transposes anywhere.

Precision: bf16 matmul inputs, f32 PSUM/residual/LN math. Softmax without
max subtraction (scores bounded ~[-2.3, 2.7] for this model's init scale).
Causality via per-core additive mask inputs (SPMD uniform program).
"""

import os
import sys

import numpy as np
import ml_dtypes

sys.path.insert(0, "/opt/trn_rl_repo")

import concourse.bass as bass  # noqa: E402
import concourse.mybir as mybir  # noqa: E402
import concourse.tile as tile  # noqa: E402
from concourse import bacc  # noqa: E402
from concourse.bass_utils import run_bass_kernel_spmd  # noqa: E402

BF16 = mybir.dt.bfloat16
F32 = mybir.dt.float32
AF = mybir.ActivationFunctionType
ALU = mybir.AluOpType

P = 128
E = 768
EC = E // P  # 6
H = 12
HS = 64
B = 2
T = 1024
NB = T // P  # 8 blocks per batch
TOK = 256  # tokens per core
L_FULL = 12
V = 50257
NCORE = 8
VSH = 6283  # per-core vocab shard (8*6283 = 50264 >= V)
EPS = 1e-5

RG4 = [[0, 1, 2, 3], [4, 5, 6, 7]]
RG8 = [[0, 1, 2, 3, 4, 5, 6, 7]]

_nbf = ml_dtypes.bfloat16


def _build(L, VS, no_ag=False):
    nc = bacc.Bacc("TRN2", target_bir_lowering=False, debug=False, num_devices=NCORE)

    # ---- DRAM I/O ----
    x0T_d = nc.dram_tensor("x0T", [E, TOK], F32, kind="ExternalInput").ap()
    ln1_d = nc.dram_tensor("ln1p", [L, 2, P, EC], F32, kind="ExternalInput").ap()
    ln2_d = nc.dram_tensor("ln2p", [L, 2, P, EC], F32, kind="ExternalInput").ap()
    lnf_d = nc.dram_tensor("lnfp", [2, P, EC], F32, kind="ExternalInput").ap()
    qkb_d = nc.dram_tensor("qkb", [L, P, 12], F32, kind="ExternalInput").ap()
    vb_d = nc.dram_tensor("vb", [L, E], F32, kind="ExternalInput").ap()
    pb_d = nc.dram_tensor("pb", [L, P, EC], F32, kind="ExternalInput").ap()
    fcb_d = nc.dram_tensor("fcb", [L, P, 24], F32, kind="ExternalInput").ap()
    fpb_d = nc.dram_tensor("fpb", [L, P, EC], F32, kind="ExternalInput").ap()
    aw_d = nc.dram_tensor("aw", [L, E, 3 * E], BF16, kind="ExternalInput").ap()
    pw_d = nc.dram_tensor("pw", [L, E, E], BF16, kind="ExternalInput").ap()
    fw_d = nc.dram_tensor("fw", [L, E, 4 * E], BF16, kind="ExternalInput").ap()
    fpw_d = nc.dram_tensor("fpw", [L, 4 * E, E], BF16, kind="ExternalInput").ap()
    wteT_d = nc.dram_tensor("wteT", [E, VS], BF16, kind="ExternalInput").ap()
    msk_d = nc.dram_tensor("msk", [NB, P, TOK], F32, kind="ExternalInput").ap()
    out_d = nc.dram_tensor("out", [NCORE * TOK, VS], F32, kind="ExternalOutput").ap()

    with tile.TileContext(nc) as tc:
        with (
            tc.tile_pool(name="persist", bufs=1) as persist,
            tc.tile_pool(name="resid", bufs=1) as resid,
            tc.tile_pool(name="lnp", bufs=2) as lnp,
            tc.tile_pool(name="wpool", bufs=3) as wpool,
            tc.tile_pool(name="actp", bufs=2) as actp,
            tc.tile_pool(name="qkvp", bufs=1) as qkvp,
            tc.tile_pool(name="htp", bufs=1) as htp,
            tc.tile_pool(name="kvg", bufs=1) as kvg,
            tc.tile_pool(name="attp", bufs=10) as attp,
            tc.tile_pool(name="smallp", bufs=4) as smallp,
            tc.tile_pool(name="wtp", bufs=12) as wtp,
            tc.tile_pool(name="lop", bufs=4) as lop,
            tc.tile_pool(name="dram", bufs=2, space="DRAM") as dram,
            tc.tile_pool(name="ps_big", bufs=6, space="PSUM") as ps_big,
            tc.tile_pool(name="ps_sml", bufs=2, space="PSUM") as ps_sml,
        ):
            # ---- persistent constants ----
            ones_col_f = persist.tile([P, 1], F32, name="ones_col_f")
            nc.vector.memset(ones_col_f, 1.0)
            ones_col = persist.tile([P, 1], BF16, name="ones_col")
            nc.vector.tensor_copy(ones_col, ones_col_f)
            ones_row_f = persist.tile([1, P], F32, name="ones_row_f")
            nc.vector.memset(ones_row_f, 1.0)
            ones_row = persist.tile([1, P], BF16, name="ones_row")
            nc.vector.tensor_copy(ones_row, ones_row_f)
            eps_sb = persist.tile([P, 1], F32, name="eps_sb")
            nc.vector.memset(eps_sb, EPS)
            mask_sb = persist.tile([P, NB, TOK], F32, name="mask_sb")
            for j in range(NB):
                nc.sync.dma_start(mask_sb[:, j, :], msk_d[j])
            lnfw = persist.tile([P, EC], F32, name="lnfw")
            lnfb = persist.tile([P, EC], F32, name="lnfb")
            nc.sync.dma_start(lnfw, lnf_d[0])
            nc.sync.dma_start(lnfb, lnf_d[1])

            # ---- residual (lives whole kernel) ----
            xT = resid.tile([P, EC, TOK], F32, name="xT")
            for c in range(EC):
                nc.sync.dma_start(xT[:, c, :], x0T_d[c * P : (c + 1) * P, :])

            def layer_norm(w_sb, b_sb, tagp):
                """xT (f32, resid) -> new bf16 tile [P, EC, TOK]."""
                s1 = ps_sml.tile([1, TOK], F32, name=f"s1{tagp}", tag="pss")
                s2 = ps_sml.tile([1, TOK], F32, name=f"s2{tagp}", tag="pss")
                for c in range(EC):
                    xb = actp.tile([P, TOK], BF16, name=f"xb{tagp}", tag="xb")
                    sq = actp.tile([P, TOK], BF16, name=f"sq{tagp}", tag="sq")
                    nc.vector.tensor_copy(xb, xT[:, c, :])
                    nc.vector.tensor_tensor(sq, xT[:, c, :], xT[:, c, :], ALU.mult)
                    nc.tensor.matmul(s1, ones_col, xb, start=(c == 0), stop=(c == EC - 1))
                    nc.tensor.matmul(s2, ones_col, sq, start=(c == 0), stop=(c == EC - 1))
                mean = smallp.tile([1, TOK], F32, name=f"mean{tagp}", tag="sm1")
                ex2 = smallp.tile([1, TOK], F32, name=f"ex2{tagp}", tag="sm2")
                nc.scalar.mul(mean, s1, 1.0 / E)
                nc.scalar.mul(ex2, s2, 1.0 / E)
                var = smallp.tile([1, TOK], F32, name=f"var{tagp}", tag="sm3")
                nc.vector.tensor_tensor(var, mean, mean, ALU.mult)
                nc.vector.tensor_tensor(var, ex2, var, ALU.subtract)
                std = smallp.tile([1, TOK], F32, name=f"std{tagp}", tag="sm4")
                nc.scalar.activation(std, var, AF.Sqrt, bias=eps_sb[0:1])
                rstd = smallp.tile([1, TOK], F32, name=f"rstd{tagp}", tag="sm5")
                nc.vector.reciprocal(rstd, std)
                rstd_b = smallp.tile([1, TOK], BF16, name=f"rstdb{tagp}", tag="sm6")
                nc.vector.tensor_copy(rstd_b, rstd)
                mr_b = smallp.tile([1, TOK], BF16, name=f"mrb{tagp}", tag="sm7")
                nc.vector.tensor_tensor(mr_b, mean, rstd, ALU.mult)
                a_ps = ps_sml.tile([P, TOK], F32, name=f"aps{tagp}", tag="pss")
                c_ps = ps_sml.tile([P, TOK], F32, name=f"cps{tagp}", tag="pss")
                nc.tensor.matmul(a_ps, ones_row, rstd_b, start=True, stop=True)
                nc.tensor.matmul(c_ps, ones_row, mr_b, start=True, stop=True)
                a_sb = actp.tile([P, TOK], F32, name=f"asb{tagp}", tag="asb")
                c_sb = actp.tile([P, TOK], F32, name=f"csb{tagp}", tag="csb")
                nc.vector.tensor_copy(a_sb, a_ps)
                nc.vector.tensor_copy(c_sb, c_ps)
                ln = actp.tile([P, EC, TOK], BF16, name=f"ln{tagp}", tag=f"ln{tagp}")
                for c in range(EC):
                    tmp = actp.tile([P, TOK], F32, name=f"lt{tagp}", tag="lntmp")
                    nc.vector.tensor_tensor(tmp, xT[:, c, :], a_sb, ALU.mult)
                    nc.vector.tensor_tensor(tmp, tmp, c_sb, ALU.subtract)
                    nc.scalar.activation(
                        ln[:, c, :], tmp, AF.Identity,
                        bias=b_sb[:, c : c + 1], scale=w_sb[:, c : c + 1],
                    )
                return ln

            for l in range(L):
                # ---- layer params ----
                l1w = lnp.tile([P, EC], F32, name=f"l1w{l}", tag="l1w")
                l1b = lnp.tile([P, EC], F32, name=f"l1b{l}", tag="l1b")
                nc.sync.dma_start(l1w, ln1_d[l, 0])
                nc.sync.dma_start(l1b, ln1_d[l, 1])

                ln1 = layer_norm(l1w, l1b, "a")

                # ---- qT / kT (feature-major, bf16) ----
                qkb = lnp.tile([P, 12], F32, name=f"qkb{l}", tag="qkb")
                nc.sync.dma_start(qkb, qkb_d[l])
                qT = qkvp.tile([P, EC, TOK], BF16, name=f"qT{l}", tag="qT")
                kTo = qkvp.tile([P, EC, TOK], BF16, name=f"kTo{l}", tag="kTo")

                def qk_group(grp, dst):
                    pss = [
                        ps_big.tile([P, TOK], F32, name=f"qk{l}_{grp}_{t}", tag="ps")
                        for t in range(EC)
                    ]
                    for c in range(EC):
                        awc = wpool.tile([P, E], BF16, name=f"aw{l}_{grp}_{c}", tag="aw")
                        nc.sync.dma_start(
                            awc, aw_d[l, c * P : (c + 1) * P, grp * E : (grp + 1) * E]
                        )
                        for t in range(EC):
                            nc.tensor.matmul(
                                pss[t], awc[:, t * P : (t + 1) * P], ln1[:, c, :],
                                start=(c == 0), stop=(c == EC - 1),
                            )
                    for t in range(EC):
                        nc.scalar.activation(
                            dst[:, t, :], pss[t], AF.Identity,
                            bias=qkb[:, grp * EC + t : grp * EC + t + 1],
                        )

                # K first so its AllGather overlaps V and Q compute
                qk_group(1, kTo)

                # ---- AllGather K early (overlaps with v compute) ----
                cink = dram.tile([E, TOK], BF16, name=f"cink{l}", tag="cink")
                for c in range(EC):
                    nc.sync.dma_start(cink[c * P : (c + 1) * P, :], kTo[:, c, :])
                coutk = dram.tile([4, E, TOK], BF16, name=f"coutk{l}", tag="coutk")
                if not no_ag:
                    nc.gpsimd.collective_compute(
                        "AllGather", ALU.bypass, replica_groups=RG4,
                        ins=[cink[:].opt()], outs=[coutk[:].opt()],
                    )

                # ---- v (token-major, bf16, bias added) ----
                vbias = lnp.tile([P, E], F32, name=f"vbias{l}", tag="vbias")
                vb_src = bass.AP(
                    tensor=vb_d.tensor, offset=vb_d[l].offset,
                    ap=[[0, P], [1, E]],
                )
                nc.sync.dma_start(vbias, vb_src)
                vo = qkvp.tile([P, 2, E], BF16, name=f"vo{l}", tag="vo")
                pvs = [
                    [
                        ps_big.tile([P, 512], F32, name=f"v{l}_{tt}a", tag="ps"),
                        ps_big.tile([P, TOK], F32, name=f"v{l}_{tt}b", tag="ps"),
                    ]
                    for tt in range(2)
                ]
                for c in range(EC):
                    awv = wpool.tile([P, E], BF16, name=f"awv{l}_{c}", tag="aw")
                    nc.sync.dma_start(
                        awv, aw_d[l, c * P : (c + 1) * P, 2 * E : 3 * E]
                    )
                    for tt in range(2):
                        lt = ln1[:, c, tt * P : (tt + 1) * P]
                        nc.tensor.matmul(
                            pvs[tt][0], lt, awv[:, 0:512],
                            start=(c == 0), stop=(c == EC - 1),
                        )
                        nc.tensor.matmul(
                            pvs[tt][1], lt, awv[:, 512:768],
                            start=(c == 0), stop=(c == EC - 1),
                        )
                for tt in range(2):
                    nc.vector.tensor_tensor(
                        vo[:, tt, 0:512], pvs[tt][0], vbias[:, 0:512], ALU.add
                    )
                    nc.vector.tensor_tensor(
                        vo[:, tt, 512:768], pvs[tt][1], vbias[:, 512:768], ALU.add
                    )

                # ---- AllGather V within 4-core group ----
                cinv = dram.tile([2 * P, E], BF16, name=f"cinv{l}", tag="cinv")
                for tt in range(2):
                    nc.sync.dma_start(cinv[tt * P : (tt + 1) * P, :], vo[:, tt, :])
                coutv = dram.tile([4, 2 * P, E], BF16, name=f"coutv{l}", tag="coutv")
                if not no_ag:
                    nc.gpsimd.collective_compute(
                        "AllGather", ALU.bypass, replica_groups=RG4,
                        ins=[cinv[:].opt()], outs=[coutv[:].opt()],
                    )

                # Q group last: both AllGathers are in flight during it
                qk_group(0, qT)

                # ---- stage gathered K/V in per-source tiles so attention
                # matmuls on a block only wait for that block's staging DMA ----
                kf_t = [
                    kvg.tile([P, EC, TOK], BF16, name=f"kf{l}_{s2}", tag=f"kf{s2}")
                    for s2 in range(4)
                ]
                va_t = [
                    kvg.tile([P, H, HS + 1], BF16, name=f"va{l}_{j}", tag=f"va{j}")
                    for j in range(NB)
                ]
                for j in range(NB):
                    nc.vector.memset(va_t[j][:, :, HS : HS + 1], 1.0)
                for src in range(4):
                    ksrc = cink if no_ag else coutk[src]
                    vsrc = cinv if no_ag else coutv[src]
                    ck = ksrc.rearrange("(c p) t -> p c t", p=P)
                    nc.sync.dma_start(kf_t[src][:], ck)
                    for tt in range(2):
                        j = 2 * src + tt
                        cvb = vsrc[tt * P : (tt + 1) * P, :].rearrange(
                            "t (h d) -> t h d", d=HS
                        )
                        nc.sync.dma_start(va_t[j][:, :, 0:HS], cvb)

                # ---- attention (uniform over all 8 k-blocks; masks are data) ----
                # Heads processed in even/odd pairs sharing an e-chunk: their
                # score matmuls use disjoint PE row strips (base 0 vs 64) and
                # execute concurrently on the array.
                yT = qkvp.tile([P, EC, TOK], BF16, name=f"yT{l}", tag="yT")
                for hp in range(H // 2):
                    ch = hp
                    Es = {0: [], 1: []}
                    for j in range(NB):
                        scs = {}
                        for s_ in range(2):
                            po = s_ * HS
                            sc = ps_big.tile(
                                [P, TOK], F32, name=f"sc{l}_{hp}_{j}_{s_}", tag="ps"
                            )
                            nc.tensor.matmul(
                                sc,
                                kf_t[j // 2][
                                    po : po + HS, ch, (j % 2) * P : (j % 2 + 1) * P
                                ],
                                qT[po : po + HS, ch, :],
                                start=True, stop=True,
                            )
                            scs[s_] = sc
                        for s_ in range(2):
                            sc = scs[s_]
                            nc.vector.tensor_tensor(sc, sc, mask_sb[:, j, :], ALU.add)
                            Ej = attp.tile(
                                [P, TOK], BF16, name=f"E{l}_{hp}_{j}_{s_}", tag="E"
                            )
                            nc.scalar.activation(Ej, sc, AF.Exp)
                            Es[s_].append(Ej)
                    for s_ in range(2):
                        h = 2 * hp + s_
                        po = s_ * HS
                        y_ps = ps_sml.tile([P, TOK], F32, name=f"y{l}_{h}", tag="pss")
                        for j in range(NB):
                            nc.tensor.matmul(
                                y_ps[0 : HS + 1, :], va_t[j][:, h, :], Es[s_][j],
                                start=(j == 0), stop=(j == NB - 1),
                            )
                        zinv = smallp.tile([1, TOK], F32, name=f"zi{l}_{h}", tag="zi")
                        nc.vector.reciprocal(zinv, y_ps[HS : HS + 1, :])
                        zinv_b = smallp.tile([1, TOK], BF16, name=f"zib{l}_{h}", tag="zib")
                        nc.vector.tensor_copy(zinv_b, zinv)
                        z_ps = ps_sml.tile([HS, TOK], F32, name=f"zp{l}_{h}", tag="pss")
                        nc.tensor.matmul(
                            z_ps, ones_row[:, 0:HS], zinv_b, start=True, stop=True
                        )
                        zb = attp.tile([HS, TOK], F32, name=f"zb{l}_{h}", tag="zb")
                        nc.scalar.copy(zb, z_ps)
                        nc.vector.tensor_tensor(
                            yT[po : po + HS, ch, :], y_ps[0:HS, :], zb, ALU.mult
                        )

                # ---- attn proj + residual ----
                pbt = lnp.tile([P, EC], F32, name=f"pbt{l}", tag="pbt")
                nc.sync.dma_start(pbt, pb_d[l])
                pss = [
                    ps_big.tile([P, TOK], F32, name=f"pj{l}_{t}", tag="ps")
                    for t in range(EC)
                ]
                for c in range(EC):
                    pwc = wpool.tile([P, E], BF16, name=f"pw{l}_{c}", tag="pw")
                    nc.sync.dma_start(pwc, pw_d[l, c * P : (c + 1) * P, :])
                    for t in range(EC):
                        nc.tensor.matmul(
                            pss[t], pwc[:, t * P : (t + 1) * P], yT[:, c, :],
                            start=(c == 0), stop=(c == EC - 1),
                        )
                for t in range(EC):
                    tmp = actp.tile([P, TOK], F32, name=f"pe{l}_{t}", tag="ep")
                    nc.scalar.activation(
                        tmp, pss[t], AF.Identity, bias=pbt[:, t : t + 1]
                    )
                    nc.vector.tensor_tensor(
                        xT[:, t, :], xT[:, t, :], tmp, ALU.add
                    )

                # ---- MLP ----
                l2w = lnp.tile([P, EC], F32, name=f"l2w{l}", tag="l2w")
                l2b = lnp.tile([P, EC], F32, name=f"l2b{l}", tag="l2b")
                nc.sync.dma_start(l2w, ln2_d[l, 0])
                nc.sync.dma_start(l2b, ln2_d[l, 1])
                ln2 = layer_norm(l2w, l2b, "b")

                fcb = lnp.tile([P, 24], F32, name=f"fcb{l}", tag="fcb")
                nc.sync.dma_start(fcb, fcb_d[l])
                hT = htp.tile([P, 24, TOK], BF16, name=f"hT{l}", tag="hT")
                for grp in range(4):
                    pss = [
                        ps_big.tile([P, TOK], F32, name=f"fc{l}_{grp}_{t}", tag="ps")
                        for t in range(EC)
                    ]
                    for c in range(EC):
                        fwc = wpool.tile([P, E], BF16, name=f"fw{l}_{grp}_{c}", tag="fw")
                        nc.sync.dma_start(
                            fwc, fw_d[l, c * P : (c + 1) * P, grp * E : (grp + 1) * E]
                        )
                        for t in range(EC):
                            nc.tensor.matmul(
                                pss[t], fwc[:, t * P : (t + 1) * P], ln2[:, c, :],
                                start=(c == 0), stop=(c == EC - 1),
                            )
                    for t in range(EC):
                        col = grp * EC + t
                        nc.scalar.activation(
                            hT[:, col, :], pss[t], AF.Gelu,
                            bias=fcb[:, col : col + 1],
                        )

                fpb = lnp.tile([P, EC], F32, name=f"fpb{l}", tag="fpb")
                nc.sync.dma_start(fpb, fpb_d[l])
                pss = [
                    ps_big.tile([P, TOK], F32, name=f"fp{l}_{t}", tag="ps")
                    for t in range(EC)
                ]
                for hc in range(24):
                    fpc = wpool.tile([P, E], BF16, name=f"fpw{l}_{hc}", tag="fpw")
                    nc.sync.dma_start(fpc, fpw_d[l, hc * P : (hc + 1) * P, :])
                    for t in range(EC):
                        nc.tensor.matmul(
                            pss[t], fpc[:, t * P : (t + 1) * P], hT[:, hc, :],
                            start=(hc == 0), stop=(hc == 23),
                        )
                for t in range(EC):
                    tmp = actp.tile([P, TOK], F32, name=f"fe{l}_{t}", tag="ep")
                    nc.scalar.activation(
                        tmp, pss[t], AF.Identity, bias=fpb[:, t : t + 1]
                    )
                    nc.vector.tensor_tensor(
                        xT[:, t, :], xT[:, t, :], tmp, ALU.add
                    )

            # ---- final LN + 8-core AllGather ----
            lnf = layer_norm(lnfw, lnfb, "f")
            fin_in = dram.tile([1, E, TOK], BF16, name="fin_in", tag="fin_in")
            for c in range(EC):
                nc.sync.dma_start(fin_in[0][c * P : (c + 1) * P, :], lnf[:, c, :])
            fin_out = dram.tile([NCORE, E, TOK], BF16, name="fin_out", tag="fin_out", addr_space="Shared")
            if not no_ag:
                nc.gpsimd.collective_compute(
                    "AllGather", ALU.bypass, replica_groups=RG8,
                    ins=[fin_in[:].opt()], outs=[fin_out[:].opt()],
                )
            lnfall = persist.tile([P, EC, NCORE * TOK], BF16, name="lnfall")
            for src in range(NCORE):
                csrc = (fin_in[0] if no_ag else fin_out[src]).rearrange("(c p) t -> p c t", p=P)
                nc.sync.dma_start(
                    lnfall[:, :, src * TOK : (src + 1) * TOK], csrc
                )

            # ---- lm_head: logits[tok, v] for ALL tokens x own vocab shard ----
            nvt = (VS + 511) // 512
            ntt = NCORE * TOK // P  # 16
            for vt in range(nvt):
                w = min(512, VS - vt * 512)
                wts = []
                for c in range(EC):
                    wtc = wtp.tile([P, 512], BF16, name=f"wt{vt}_{c}", tag="wt")
                    nc.sync.dma_start(
                        wtc[:, :w], wteT_d[c * P : (c + 1) * P, vt * 512 : vt * 512 + w]
                    )
                    wts.append(wtc)
                for tt in range(ntt):
                    lps = ps_big.tile([P, 512], F32, name=f"lm{vt}_{tt}", tag="ps")
                    for c in range(EC):
                        nc.tensor.matmul(
                            lps[:, :w],
                            lnfall[:, c, tt * P : (tt + 1) * P],
                            wts[c][:, :w],
                            start=(c == 0), stop=(c == EC - 1),
                        )
                    o = lop.tile([P, 512], F32, name=f"lo{vt}_{tt}", tag="lo")
                    if tt % 2 == 0:
                        nc.scalar.copy(o[:, :w], lps[:, :w])
                    else:
                        nc.vector.tensor_copy(o[:, :w], lps[:, :w])
                    nc.sync.dma_start(
                        out_d[tt * P : (tt + 1) * P, vt * 512 : vt * 512 + w],
                        o[:, :w],
                    )

    nc.compile()
    return nc


_CACHE = {}


def _get_nc(L, VS, no_ag=False):
    key = (L, VS, no_ag)
    if key not in _CACHE:
        _CACHE[key] = _build(L, VS, no_ag=no_ag)
    return _CACHE[key]


def _bf(a):
    return np.ascontiguousarray(a.astype(_nbf))


def _pp(a, cols):
    """[L?, n*128] feature vector -> per-partition layout [..., 128, n]."""
    a = np.asarray(a, np.float32)
    shp = a.shape[:-1]
    n = a.shape[-1] // P
    return np.ascontiguousarray(
        a.reshape(*shp, n, P).swapaxes(-1, -2)
    )


def _prepare(inputs, L, VS):
    """Host prep: embedding, weight cast/fold/transpose, per-core in_maps."""
    idx = np.asarray(inputs["idx"])
    wte = np.asarray(inputs["wte"], np.float32)
    wpe = np.asarray(inputs["wpe"], np.float32)

    x0 = wte[idx] + wpe[None, :T]  # [B, T, E] f32

    attn_w = np.asarray(inputs["attn_w"], np.float32)[:L].copy()
    attn_b = np.asarray(inputs["attn_b"], np.float32)[:L].copy()
    scale = 1.0 / np.sqrt(HS)
    attn_w[:, :, :E] *= scale
    attn_b[:, :E] *= scale

    aw = _bf(attn_w)
    pw = _bf(np.asarray(inputs["proj_w"], np.float32)[:L])
    fw = _bf(np.asarray(inputs["fc_w"], np.float32)[:L])
    fpw = _bf(np.asarray(inputs["fcp_w"], np.float32)[:L])

    ln1p = np.stack(
        [_pp(np.asarray(inputs["ln1_w"], np.float32)[:L], EC),
         _pp(np.asarray(inputs["ln1_b"], np.float32)[:L], EC)], axis=1
    )
    ln2p = np.stack(
        [_pp(np.asarray(inputs["ln2_w"], np.float32)[:L], EC),
         _pp(np.asarray(inputs["ln2_b"], np.float32)[:L], EC)], axis=1
    )
    lnfp = np.stack(
        [_pp(np.asarray(inputs["lnf_w"], np.float32), EC),
         _pp(np.asarray(inputs["lnf_b"], np.float32), EC)], axis=0
    )
    qkb = _pp(attn_b[:, : 2 * E], 12)
    vb = np.ascontiguousarray(attn_b[:, 2 * E :])
    pb = _pp(np.asarray(inputs["proj_b"], np.float32)[:L], EC)
    fcb = _pp(np.asarray(inputs["fc_b"], np.float32)[:L], 24)
    fpb = _pp(np.asarray(inputs["fcp_b"], np.float32)[:L], EC)

    # wteT padded + per-core vocab shards
    wteT = np.zeros((E, NCORE * VS), _nbf)
    nv = min(V, NCORE * VS)
    wteT[:, :nv] = _bf(wte.T[:, :nv])

    in_maps = []
    for c in range(NCORE):
        b = c // 4
        g = c % 4
        t0 = g * TOK  # tokens [t0, t0+256) of batch b
        x0T = np.ascontiguousarray(x0[b, t0 : t0 + TOK, :].T)  # [768, 256]
        # causal masks: scoresT block [k-block j, 128k x 256q]
        msk = np.zeros((NB, P, TOK), np.float32)
        kpos = np.arange(P)
        qpos = t0 + np.arange(TOK)
        for j in range(NB):
            valid = (j * P + kpos)[:, None] <= qpos[None, :]
            msk[j] = np.where(valid, 0.0, -1e9)
        in_maps.append(
            {
                "x0T": x0T,
                "ln1p": ln1p, "ln2p": ln2p, "lnfp": lnfp,
                "qkb": qkb, "vb": vb, "pb": pb, "fcb": fcb, "fpb": fpb,
                "aw": aw, "pw": pw, "fw": fw, "fpw": fpw,
                "wteT": np.ascontiguousarray(wteT[:, c * VS : (c + 1) * VS]),
                "msk": msk,
            }
        )
    return in_maps


def _run(inputs, L, VS, trace=False):
    nc = _get_nc(L, VS)
    in_maps = _prepare(inputs, L, VS)
    res = run_bass_kernel_spmd(
        nc, in_maps, core_ids=list(range(NCORE)), trace=trace
    )
    # out[c] is [2048, VS] token-major (batch0 tokens then batch1); tokens of
    # batch b block-ordered by source core: src covers tokens [src%4*256 ...)
    outs = [res.results[c]["out"] for c in range(NCORE)]
    logits = np.concatenate(outs, axis=1)  # [2048, 8*VS]
    logits = logits.reshape(B, T, NCORE * VS)[:, :, :V]
    return np.ascontiguousarray(logits), res


def kernel(**inputs) -> np.ndarray:
    trace = bool(os.environ.get("_KERNEL_TRACE"))
    logits, _ = _run(inputs, L_FULL, VSH, trace=trace)
    return logits


if __name__ == "__main__":
    pass



# revision 12
# speedup vs baseline: 13.3099x; 13.3099x over previous
"""GPT-2 small forward pass on 8 Trainium2 NeuronCores (Bass/Tile).

Sharding: sequence-parallel trunk (cores 0-3 batch 0, cores 4-7 batch 1;
core owns 2 contiguous 128-token blocks), per-layer AllGather of K/V (bf16)
within each 4-core group, then vocab-sharded lm_head (8 ways) after an
8-core AllGather of the final LN output.

Layout: activations feature-major xT [768, tok] on-chip; scores computed
transposed (K stationary), PV uses token-major V with an appended ones
column so softmax denominators fall out of the same matmul; Z division is
folded into the PV epilogue. No PE transposes anywhere.

Precision: bf16 matmul inputs, f32 PSUM/residual/LN math. Softmax without
max subtraction (scores bounded ~[-2.3, 2.7] for this model's init scale).
Causality via per-core additive mask inputs (SPMD uniform program).
"""

import os
import sys

import numpy as np
import ml_dtypes

sys.path.insert(0, "/opt/trn_rl_repo")

import concourse.bass as bass  # noqa: E402
import concourse.mybir as mybir  # noqa: E402
import concourse.tile as tile  # noqa: E402
from concourse import bacc  # noqa: E402
from concourse.bass_utils import run_bass_kernel_spmd  # noqa: E402

BF16 = mybir.dt.bfloat16
F32 = mybir.dt.float32
AF = mybir.ActivationFunctionType
ALU = mybir.AluOpType

P = 128
E = 768
EC = E // P  # 6
H = 12
HS = 64
B = 2
T = 1024
NB = T // P  # 8 blocks per batch
TOK = 256  # tokens per core
L_FULL = 12
V = 50257
NCORE = 8
VSH = 6283  # per-core vocab shard (8*6283 = 50264 >= V)
EPS = 1e-5

RG4 = [[0, 1, 2, 3], [4, 5, 6, 7]]
RG8 = [[0, 1, 2, 3, 4, 5, 6, 7]]

_nbf = ml_dtypes.bfloat16


def _build(L, VS, no_ag=False):
    nc = bacc.Bacc("TRN2", target_bir_lowering=False, debug=False, num_devices=NCORE)

    # ---- DRAM I/O ----
    # ln1/ln2 affine folded into aw/fw weights host-side; only lnf kept.
    x0T_d = nc.dram_tensor("x0T", [E, TOK], F32, kind="ExternalInput").ap()
    lnf_d = nc.dram_tensor("lnfp", [2, P, EC], F32, kind="ExternalInput").ap()
    qkb_d = nc.dram_tensor("qkb", [L, P, 12], F32, kind="ExternalInput").ap()
    vb_d = nc.dram_tensor("vb", [L, E], F32, kind="ExternalInput").ap()
    pb_d = nc.dram_tensor("pb", [L, P, EC], F32, kind="ExternalInput").ap()
    fcb_d = nc.dram_tensor("fcb", [L, P, 24], F32, kind="ExternalInput").ap()
    fpb_d = nc.dram_tensor("fpb", [L, P, EC], F32, kind="ExternalInput").ap()
    aw_d = nc.dram_tensor("aw", [L, E, 3 * E], BF16, kind="ExternalInput").ap()
    pw_d = nc.dram_tensor("pw", [L, E, E], BF16, kind="ExternalInput").ap()
    fw_d = nc.dram_tensor("fw", [L, E, 4 * E], BF16, kind="ExternalInput").ap()
    fpw_d = nc.dram_tensor("fpw", [L, 4 * E, E], BF16, kind="ExternalInput").ap()
    wteT_d = nc.dram_tensor("wteT", [E, VS], BF16, kind="ExternalInput").ap()
    msk_d = nc.dram_tensor("msk", [NB, P, TOK], F32, kind="ExternalInput").ap()
    out_d = nc.dram_tensor("out", [NCORE * TOK, VS], F32, kind="ExternalOutput").ap()

    with tile.TileContext(nc) as tc:
        with (
            tc.tile_pool(name="persist", bufs=1) as persist,
            tc.tile_pool(name="resid", bufs=1) as resid,
            tc.tile_pool(name="lnp", bufs=2) as lnp,
            tc.tile_pool(name="wpool", bufs=3) as wpool,
            tc.tile_pool(name="actp", bufs=2) as actp,
            tc.tile_pool(name="qkvp", bufs=1) as qkvp,
            tc.tile_pool(name="htp", bufs=1) as htp,
            tc.tile_pool(name="kvg", bufs=1) as kvg,
            tc.tile_pool(name="attp", bufs=10) as attp,
            tc.tile_pool(name="smallp", bufs=4) as smallp,
            tc.tile_pool(name="wtp", bufs=12) as wtp,
            tc.tile_pool(name="lop", bufs=4) as lop,
            tc.tile_pool(name="dram", bufs=2, space="DRAM") as dram,
            tc.tile_pool(name="ps_big", bufs=6, space="PSUM") as ps_big,
            tc.tile_pool(name="ps_sml", bufs=2, space="PSUM") as ps_sml,
        ):
            # ---- persistent constants ----
            ones_col_f = persist.tile([P, 1], F32, name="ones_col_f")
            nc.vector.memset(ones_col_f, 1.0)
            ones_col = persist.tile([P, 1], BF16, name="ones_col")
            nc.vector.tensor_copy(ones_col, ones_col_f)
            ones_row_f = persist.tile([1, P], F32, name="ones_row_f")
            nc.vector.memset(ones_row_f, 1.0)
            ones_row = persist.tile([1, P], BF16, name="ones_row")
            nc.vector.tensor_copy(ones_row, ones_row_f)
            eps_sb = persist.tile([P, 1], F32, name="eps_sb")
            nc.vector.memset(eps_sb, EPS)
            mask_sb = persist.tile([P, NB, TOK], F32, name="mask_sb")
            for j in range(NB):
                nc.sync.dma_start(mask_sb[:, j, :], msk_d[j])
            lnfw = persist.tile([P, EC], F32, name="lnfw")
            lnfb = persist.tile([P, EC], F32, name="lnfb")
            nc.sync.dma_start(lnfw, lnf_d[0])
            nc.sync.dma_start(lnfb, lnf_d[1])

            # ---- residual (lives whole kernel) ----
            xT = resid.tile([P, EC, TOK], F32, name="xT")
            for c in range(EC):
                nc.sync.dma_start(xT[:, c, :], x0T_d[c * P : (c + 1) * P, :])

            def layer_norm(tagp, w_sb=None, b_sb=None):
                """xT (f32, resid) -> new bf16 tile [P, EC, TOK].

                Without w_sb/b_sb, emits the *unscaled* normalized value
                (x - m) * rstd — the affine is folded into the consuming
                weights host-side."""
                s1 = ps_sml.tile([1, TOK], F32, name=f"s1{tagp}", tag="pss")
                s2 = ps_sml.tile([1, TOK], F32, name=f"s2{tagp}", tag="pss")
                for c in range(EC):
                    xb = actp.tile([P, TOK], BF16, name=f"xb{tagp}", tag="xb")
                    sq = actp.tile([P, TOK], BF16, name=f"sq{tagp}", tag="sq")
                    nc.vector.tensor_copy(xb, xT[:, c, :])
                    nc.vector.tensor_tensor(sq, xT[:, c, :], xT[:, c, :], ALU.mult)
                    nc.tensor.matmul(s1, ones_col, xb, start=(c == 0), stop=(c == EC - 1))
                    nc.tensor.matmul(s2, ones_col, sq, start=(c == 0), stop=(c == EC - 1))
                mean = smallp.tile([1, TOK], F32, name=f"mean{tagp}", tag="sm1")
                ex2 = smallp.tile([1, TOK], F32, name=f"ex2{tagp}", tag="sm2")
                nc.scalar.mul(mean, s1, 1.0 / E)
                nc.scalar.mul(ex2, s2, 1.0 / E)
                var = smallp.tile([1, TOK], F32, name=f"var{tagp}", tag="sm3")
                nc.vector.tensor_tensor(var, mean, mean, ALU.mult)
                nc.vector.tensor_tensor(var, ex2, var, ALU.subtract)
                std = smallp.tile([1, TOK], F32, name=f"std{tagp}", tag="sm4")
                nc.scalar.activation(std, var, AF.Sqrt, bias=eps_sb[0:1])
                rstd = smallp.tile([1, TOK], F32, name=f"rstd{tagp}", tag="sm5")
                nc.vector.reciprocal(rstd, std)
                rstd_b = smallp.tile([1, TOK], BF16, name=f"rstdb{tagp}", tag="sm6")
                nc.vector.tensor_copy(rstd_b, rstd)
                mr_b = smallp.tile([1, TOK], BF16, name=f"mrb{tagp}", tag="sm7")
                nc.vector.tensor_tensor(mr_b, mean, rstd, ALU.mult)
                a_ps = ps_sml.tile([P, TOK], F32, name=f"aps{tagp}", tag="pss")
                c_ps = ps_sml.tile([P, TOK], F32, name=f"cps{tagp}", tag="pss")
                nc.tensor.matmul(a_ps, ones_row, rstd_b, start=True, stop=True)
                nc.tensor.matmul(c_ps, ones_row, mr_b, start=True, stop=True)
                a_sb = actp.tile([P, TOK], F32, name=f"asb{tagp}", tag="asb")
                c_sb = actp.tile([P, TOK], F32, name=f"csb{tagp}", tag="csb")
                nc.vector.tensor_copy(a_sb, a_ps)
                nc.vector.tensor_copy(c_sb, c_ps)
                ln = actp.tile([P, EC, TOK], BF16, name=f"ln{tagp}", tag=f"ln{tagp}")
                for c in range(EC):
                    if w_sb is None:
                        tmp = actp.tile([P, TOK], F32, name=f"lt{tagp}", tag="lntmp")
                        nc.vector.tensor_tensor(tmp, xT[:, c, :], a_sb, ALU.mult)
                        nc.vector.tensor_tensor(ln[:, c, :], tmp, c_sb, ALU.subtract)
                    else:
                        tmp = actp.tile([P, TOK], F32, name=f"lt{tagp}", tag="lntmp")
                        nc.vector.tensor_tensor(tmp, xT[:, c, :], a_sb, ALU.mult)
                        nc.vector.tensor_tensor(tmp, tmp, c_sb, ALU.subtract)
                        nc.scalar.activation(
                            ln[:, c, :], tmp, AF.Identity,
                            bias=b_sb[:, c : c + 1], scale=w_sb[:, c : c + 1],
                        )
                return ln

            for l in range(L):
                ln1 = layer_norm("a")

                # ---- qT / kT (feature-major, bf16) ----
                qkb = lnp.tile([P, 12], F32, name=f"qkb{l}", tag="qkb")
                nc.sync.dma_start(qkb, qkb_d[l])
                qT = qkvp.tile([P, EC, TOK], BF16, name=f"qT{l}", tag="qT")
                kTo = qkvp.tile([P, EC, TOK], BF16, name=f"kTo{l}", tag="kTo")

                def qk_group(grp, dst):
                    pss = [
                        ps_big.tile([P, TOK], F32, name=f"qk{l}_{grp}_{t}", tag="ps")
                        for t in range(EC)
                    ]
                    for c in range(EC):
                        awc = wpool.tile([P, E], BF16, name=f"aw{l}_{grp}_{c}", tag="aw")
                        nc.sync.dma_start(
                            awc, aw_d[l, c * P : (c + 1) * P, grp * E : (grp + 1) * E]
                        )
                        for t in range(EC):
                            nc.tensor.matmul(
                                pss[t], awc[:, t * P : (t + 1) * P], ln1[:, c, :],
                                start=(c == 0), stop=(c == EC - 1),
                            )
                    for t in range(EC):
                        nc.scalar.activation(
                            dst[:, t, :], pss[t], AF.Identity,
                            bias=qkb[:, grp * EC + t : grp * EC + t + 1],
                        )

                # K first, then V, then ONE fused K+V AllGather, then Q
                # (the single collective is in flight during Q compute).
                qk_group(1, kTo)

                # fused K+V collective buffer: [0] = K [E, TOK] feature-major,
                # [1] = V [2P, E] token-major (same element count per half).
                cinkv = dram.tile([2, E, TOK], BF16, name=f"cinkv{l}", tag="cinkv")
                cink = cinkv[0]
                cinv = cinkv[1].rearrange("e t -> (e t)").rearrange(
                    "(t e) -> t e", e=E
                )
                for c in range(EC):
                    nc.sync.dma_start(cink[c * P : (c + 1) * P, :], kTo[:, c, :])

                # ---- v (token-major, bf16, bias added) ----
                vbias = lnp.tile([P, E], F32, name=f"vbias{l}", tag="vbias")
                vb_src = bass.AP(
                    tensor=vb_d.tensor, offset=vb_d[l].offset,
                    ap=[[0, P], [1, E]],
                )
                nc.sync.dma_start(vbias, vb_src)
                vo = qkvp.tile([P, 2, E], BF16, name=f"vo{l}", tag="vo")
                pvs = [
                    [
                        ps_big.tile([P, 512], F32, name=f"v{l}_{tt}a", tag="ps"),
                        ps_big.tile([P, TOK], F32, name=f"v{l}_{tt}b", tag="ps"),
                    ]
                    for tt in range(2)
                ]
                for c in range(EC):
                    awv = wpool.tile([P, E], BF16, name=f"awv{l}_{c}", tag="aw")
                    nc.sync.dma_start(
                        awv, aw_d[l, c * P : (c + 1) * P, 2 * E : 3 * E]
                    )
                    for tt in range(2):
                        lt = ln1[:, c, tt * P : (tt + 1) * P]
                        nc.tensor.matmul(
                            pvs[tt][0], lt, awv[:, 0:512],
                            start=(c == 0), stop=(c == EC - 1),
                        )
                        nc.tensor.matmul(
                            pvs[tt][1], lt, awv[:, 512:768],
                            start=(c == 0), stop=(c == EC - 1),
                        )
                for tt in range(2):
                    nc.vector.tensor_tensor(
                        vo[:, tt, 0:512], pvs[tt][0], vbias[:, 0:512], ALU.add
                    )
                    nc.vector.tensor_tensor(
                        vo[:, tt, 512:768], pvs[tt][1], vbias[:, 512:768], ALU.add
                    )

                # ---- fused K+V AllGather within 4-core group ----
                for tt in range(2):
                    nc.sync.dma_start(cinv[tt * P : (tt + 1) * P, :], vo[:, tt, :])
                coutkv = dram.tile([4, 2, E, TOK], BF16, name=f"coutkv{l}", tag="coutkv")
                if not no_ag:
                    nc.gpsimd.collective_compute(
                        "AllGather", ALU.bypass, replica_groups=RG4,
                        ins=[cinkv[:].opt()], outs=[coutkv[:].opt()],
                    )

                # Q group last: the AllGather is in flight during it
                qk_group(0, qT)

                # ---- stage gathered K/V in per-source tiles so attention
                # matmuls on a block only wait for that block's staging DMA ----
                kf_t = [
                    kvg.tile([P, EC, TOK], BF16, name=f"kf{l}_{s2}", tag=f"kf{s2}")
                    for s2 in range(4)
                ]
                va_t = [
                    kvg.tile([P, H, HS + 1], BF16, name=f"va{l}_{j}", tag=f"va{j}")
                    for j in range(NB)
                ]
                for j in range(NB):
                    nc.vector.memset(va_t[j][:, :, HS : HS + 1], 1.0)
                for src in range(4):
                    ksrc = cink if no_ag else coutkv[src, 0]
                    vsrc = cinv if no_ag else coutkv[src, 1].rearrange(
                        "e t -> (e t)"
                    ).rearrange("(t e) -> t e", e=E)
                    ck = ksrc.rearrange("(c p) t -> p c t", p=P)
                    nc.sync.dma_start(kf_t[src][:], ck)
                    for tt in range(2):
                        j = 2 * src + tt
                        cvb = vsrc[tt * P : (tt + 1) * P, :].rearrange(
                            "t (h d) -> t h d", d=HS
                        )
                        nc.sync.dma_start(va_t[j][:, :, 0:HS], cvb)

                # ---- attention (uniform over all 8 k-blocks; masks are data) ----
                # Heads processed in even/odd pairs sharing an e-chunk: their
                # score matmuls use disjoint PE row strips (base 0 vs 64) and
                # execute concurrently on the array.
                yT = qkvp.tile([P, EC, TOK], BF16, name=f"yT{l}", tag="yT")
                for hp in range(H // 2):
                    ch = hp
                    Es = {0: [], 1: []}
                    for j in range(NB):
                        scs = {}
                        for s_ in range(2):
                            po = s_ * HS
                            sc = ps_big.tile(
                                [P, TOK], F32, name=f"sc{l}_{hp}_{j}_{s_}", tag="ps"
                            )
                            nc.tensor.matmul(
                                sc,
                                kf_t[j // 2][
                                    po : po + HS, ch, (j % 2) * P : (j % 2 + 1) * P
                                ],
                                qT[po : po + HS, ch, :],
                                start=True, stop=True,
                            )
                            scs[s_] = sc
                        for s_ in range(2):
                            sc = scs[s_]
                            nc.vector.tensor_tensor(sc, sc, mask_sb[:, j, :], ALU.add)
                            Ej = attp.tile(
                                [P, TOK], BF16, name=f"E{l}_{hp}_{j}_{s_}", tag="E"
                            )
                            nc.scalar.activation(Ej, sc, AF.Exp)
                            Es[s_].append(Ej)
                    for s_ in range(2):
                        h = 2 * hp + s_
                        po = s_ * HS
                        y_ps = ps_sml.tile([P, TOK], F32, name=f"y{l}_{h}", tag="pss")
                        for j in range(NB):
                            nc.tensor.matmul(
                                y_ps[0 : HS + 1, :], va_t[j][:, h, :], Es[s_][j],
                                start=(j == 0), stop=(j == NB - 1),
                            )
                        zinv = smallp.tile([1, TOK], F32, name=f"zi{l}_{h}", tag="zi")
                        nc.vector.reciprocal(zinv, y_ps[HS : HS + 1, :])
                        zinv_b = smallp.tile([1, TOK], BF16, name=f"zib{l}_{h}", tag="zib")
                        nc.vector.tensor_copy(zinv_b, zinv)
                        z_ps = ps_sml.tile([HS, TOK], F32, name=f"zp{l}_{h}", tag="pss")
                        nc.tensor.matmul(
                            z_ps, ones_row[:, 0:HS], zinv_b, start=True, stop=True
                        )
                        zb = attp.tile([HS, TOK], F32, name=f"zb{l}_{h}", tag="zb")
                        nc.scalar.copy(zb, z_ps)
                        nc.vector.tensor_tensor(
                            yT[po : po + HS, ch, :], y_ps[0:HS, :], zb, ALU.mult
                        )

                # ---- attn proj + residual ----
                pbt = lnp.tile([P, EC], F32, name=f"pbt{l}", tag="pbt")
                nc.sync.dma_start(pbt, pb_d[l])
                pss = [
                    ps_big.tile([P, TOK], F32, name=f"pj{l}_{t}", tag="ps")
                    for t in range(EC)
                ]
                for c in range(EC):
                    pwc = wpool.tile([P, E], BF16, name=f"pw{l}_{c}", tag="pw")
                    nc.sync.dma_start(pwc, pw_d[l, c * P : (c + 1) * P, :])
                    for t in range(EC):
                        nc.tensor.matmul(
                            pss[t], pwc[:, t * P : (t + 1) * P], yT[:, c, :],
                            start=(c == 0), stop=(c == EC - 1),
                        )
                for t in range(EC):
                    tmp = actp.tile([P, TOK], F32, name=f"pe{l}_{t}", tag="ep")
                    nc.scalar.activation(
                        tmp, pss[t], AF.Identity, bias=pbt[:, t : t + 1]
                    )
                    nc.vector.tensor_tensor(
                        xT[:, t, :], xT[:, t, :], tmp, ALU.add
                    )

                # ---- MLP ----
                ln2 = layer_norm("b")

                fcb = lnp.tile([P, 24], F32, name=f"fcb{l}", tag="fcb")
                nc.sync.dma_start(fcb, fcb_d[l])
                hT = htp.tile([P, 24, TOK], BF16, name=f"hT{l}", tag="hT")
                for grp in range(4):
                    pss = [
                        ps_big.tile([P, TOK], F32, name=f"fc{l}_{grp}_{t}", tag="ps")
                        for t in range(EC)
                    ]
                    for c in range(EC):
                        fwc = wpool.tile([P, E], BF16, name=f"fw{l}_{grp}_{c}", tag="fw")
                        nc.sync.dma_start(
                            fwc, fw_d[l, c * P : (c + 1) * P, grp * E : (grp + 1) * E]
                        )
                        for t in range(EC):
                            nc.tensor.matmul(
                                pss[t], fwc[:, t * P : (t + 1) * P], ln2[:, c, :],
                                start=(c == 0), stop=(c == EC - 1),
                            )
                    for t in range(EC):
                        col = grp * EC + t
                        nc.scalar.activation(
                            hT[:, col, :], pss[t], AF.Gelu,
                            bias=fcb[:, col : col + 1],
                        )

                fpb = lnp.tile([P, EC], F32, name=f"fpb{l}", tag="fpb")
                nc.sync.dma_start(fpb, fpb_d[l])
                pss = [
                    ps_big.tile([P, TOK], F32, name=f"fp{l}_{t}", tag="ps")
                    for t in range(EC)
                ]
                for hc in range(24):
                    fpc = wpool.tile([P, E], BF16, name=f"fpw{l}_{hc}", tag="fpw")
                    nc.sync.dma_start(fpc, fpw_d[l, hc * P : (hc + 1) * P, :])
                    for t in range(EC):
                        nc.tensor.matmul(
                            pss[t], fpc[:, t * P : (t + 1) * P], hT[:, hc, :],
                            start=(hc == 0), stop=(hc == 23),
                        )
                for t in range(EC):
                    tmp = actp.tile([P, TOK], F32, name=f"fe{l}_{t}", tag="ep")
                    nc.scalar.activation(
                        tmp, pss[t], AF.Identity, bias=fpb[:, t : t + 1]
                    )
                    nc.vector.tensor_tensor(
                        xT[:, t, :], xT[:, t, :], tmp, ALU.add
                    )

            # ---- final LN + 8-core AllGather ----
            lnf = layer_norm("f", lnfw, lnfb)
            fin_in = dram.tile([1, E, TOK], BF16, name="fin_in", tag="fin_in")
            for c in range(EC):
                nc.sync.dma_start(fin_in[0][c * P : (c + 1) * P, :], lnf[:, c, :])
            fin_out = dram.tile([NCORE, E, TOK], BF16, name="fin_out", tag="fin_out", addr_space="Shared")
            if not no_ag:
                nc.gpsimd.collective_compute(
                    "AllGather", ALU.bypass, replica_groups=RG8,
                    ins=[fin_in[:].opt()], outs=[fin_out[:].opt()],
                )
            lnfall = persist.tile([P, EC, NCORE * TOK], BF16, name="lnfall")
            for src in range(NCORE):
                csrc = (fin_in[0] if no_ag else fin_out[src]).rearrange("(c p) t -> p c t", p=P)
                nc.sync.dma_start(
                    lnfall[:, :, src * TOK : (src + 1) * TOK], csrc
                )

            # ---- lm_head: logits[tok, v] for ALL tokens x own vocab shard ----
            nvt = (VS + 511) // 512
            ntt = NCORE * TOK // P  # 16
            for vt in range(nvt):
                w = min(512, VS - vt * 512)
                wts = []
                for c in range(EC):
                    wtc = wtp.tile([P, 512], BF16, name=f"wt{vt}_{c}", tag="wt")
                    nc.sync.dma_start(
                        wtc[:, :w], wteT_d[c * P : (c + 1) * P, vt * 512 : vt * 512 + w]
                    )
                    wts.append(wtc)
                for tt in range(ntt):
                    lps = ps_big.tile([P, 512], F32, name=f"lm{vt}_{tt}", tag="ps")
                    for c in range(EC):
                        nc.tensor.matmul(
                            lps[:, :w],
                            lnfall[:, c, tt * P : (tt + 1) * P],
                            wts[c][:, :w],
                            start=(c == 0), stop=(c == EC - 1),
                        )
                    o = lop.tile([P, 512], F32, name=f"lo{vt}_{tt}", tag="lo")
                    if tt % 2 == 0:
                        nc.scalar.copy(o[:, :w], lps[:, :w])
                    else:
                        nc.vector.tensor_copy(o[:, :w], lps[:, :w])
                    nc.sync.dma_start(
                        out_d[tt * P : (tt + 1) * P, vt * 512 : vt * 512 + w],
                        o[:, :w],
                    )

    nc.compile()
    return nc


_CACHE = {}


def _get_nc(L, VS, no_ag=False):
    key = (L, VS, no_ag)
    if key not in _CACHE:
        _CACHE[key] = _build(L, VS, no_ag=no_ag)
    return _CACHE[key]


def _bf(a):
    return np.ascontiguousarray(a.astype(_nbf))


def _pp(a, cols):
    """[L?, n*128] feature vector -> per-partition layout [..., 128, n]."""
    a = np.asarray(a, np.float32)
    shp = a.shape[:-1]
    n = a.shape[-1] // P
    return np.ascontiguousarray(
        a.reshape(*shp, n, P).swapaxes(-1, -2)
    )


def _prepare(inputs, L, VS):
    """Host prep: embedding, weight cast/fold/transpose, per-core in_maps."""
    idx = np.asarray(inputs["idx"])
    wte = np.asarray(inputs["wte"], np.float32)
    wpe = np.asarray(inputs["wpe"], np.float32)

    x0 = wte[idx] + wpe[None, :T]  # [B, T, E] f32

    # Fold ln1 affine into attn_w/attn_b, ln2 into fc_w/fc_b (LN in-kernel
    # then emits the unscaled normalized value).
    ln1_w = np.asarray(inputs["ln1_w"], np.float32)[:L]
    ln1_b = np.asarray(inputs["ln1_b"], np.float32)[:L]
    ln2_w = np.asarray(inputs["ln2_w"], np.float32)[:L]
    ln2_b = np.asarray(inputs["ln2_b"], np.float32)[:L]

    attn_w = np.asarray(inputs["attn_w"], np.float32)[:L].copy()
    attn_b = np.asarray(inputs["attn_b"], np.float32)[:L].copy()
    attn_b = attn_b + np.einsum("le,leo->lo", ln1_b, attn_w)
    attn_w = attn_w * ln1_w[:, :, None]
    scale = 1.0 / np.sqrt(HS)
    attn_w[:, :, :E] *= scale
    attn_b[:, :E] *= scale

    fc_w = np.asarray(inputs["fc_w"], np.float32)[:L].copy()
    fc_b = np.asarray(inputs["fc_b"], np.float32)[:L].copy()
    fc_b = fc_b + np.einsum("le,leo->lo", ln2_b, fc_w)
    fc_w = fc_w * ln2_w[:, :, None]

    aw = _bf(attn_w)
    pw = _bf(np.asarray(inputs["proj_w"], np.float32)[:L])
    fw = _bf(fc_w)
    fpw = _bf(np.asarray(inputs["fcp_w"], np.float32)[:L])

    lnfp = np.stack(
        [_pp(np.asarray(inputs["lnf_w"], np.float32), EC),
         _pp(np.asarray(inputs["lnf_b"], np.float32), EC)], axis=0
    )
    qkb = _pp(attn_b[:, : 2 * E], 12)
    vb = np.ascontiguousarray(attn_b[:, 2 * E :])
    pb = _pp(np.asarray(inputs["proj_b"], np.float32)[:L], EC)
    fcb = _pp(fc_b, 24)
    fpb = _pp(np.asarray(inputs["fcp_b"], np.float32)[:L], EC)

    # wteT padded + per-core vocab shards
    wteT = np.zeros((E, NCORE * VS), _nbf)
    nv = min(V, NCORE * VS)
    wteT[:, :nv] = _bf(wte.T[:, :nv])

    in_maps = []
    for c in range(NCORE):
        b = c // 4
        g = c % 4
        t0 = g * TOK  # tokens [t0, t0+256) of batch b
        x0T = np.ascontiguousarray(x0[b, t0 : t0 + TOK, :].T)  # [768, 256]
        # causal masks: scoresT block [k-block j, 128k x 256q]
        msk = np.zeros((NB, P, TOK), np.float32)
        kpos = np.arange(P)
        qpos = t0 + np.arange(TOK)
        for j in range(NB):
            valid = (j * P + kpos)[:, None] <= qpos[None, :]
            msk[j] = np.where(valid, 0.0, -1e9)
        in_maps.append(
            {
                "x0T": x0T,
                "lnfp": lnfp,
                "qkb": qkb, "vb": vb, "pb": pb, "fcb": fcb, "fpb": fpb,
                "aw": aw, "pw": pw, "fw": fw, "fpw": fpw,
                "wteT": np.ascontiguousarray(wteT[:, c * VS : (c + 1) * VS]),
                "msk": msk,
            }
        )
    return in_maps


def _run(inputs, L, VS, trace=False):
    nc = _get_nc(L, VS)
    in_maps = _prepare(inputs, L, VS)
    res = run_bass_kernel_spmd(
        nc, in_maps, core_ids=list(range(NCORE)), trace=trace
    )
    # out[c] is [2048, VS] token-major (batch0 tokens then batch1); tokens of
    # batch b block-ordered by source core: src covers tokens [src%4*256 ...)
    outs = [res.results[c]["out"] for c in range(NCORE)]
    logits = np.concatenate(outs, axis=1)  # [2048, 8*VS]
    logits = logits.reshape(B, T, NCORE * VS)[:, :, :V]
    return np.ascontiguousarray(logits), res


def kernel(**inputs) -> np.ndarray:
    trace = bool(os.environ.get("_KERNEL_TRACE"))
    logits, _ = _run(inputs, L_FULL, VSH, trace=trace)
    return logits


if __name__ == "__main__":
    pass

